# revision 16
# baseline (speedup 1.0000x reference)
"""Trainium2 kernel for nn_BlockLinear: gather -> per-block GEMM -> scatter-add.

Key insight: the whole op is linear in x, so gather/einsum/scatter fold into a
single dense GEMM  out[t, o] = sum_k x[t, k] * Wfull[k, o] + bias[o]  where
Wfull[k, o] = sum_{n,i,j} [input_indices[n,i]==k][output_indices[n,j]==o] * W[n,j,i].

Wfull is built on host (bincount scatter-add, exact fp64 accumulation), then the
GEMM runs on 8 NeuronCores, sharded 2D: 4 token groups x 2 out-feature groups.

Precision/speed hybrid along the contraction axis: the first NF8=12 k-tiles run
in fp8 e4m3 with the DoubleRow perf mode (2 k-tiles per PE instruction, 2x MAC
rate), the remaining 20 k-tiles in bf16 (full PE rate, half the HBM traffic of
fp32r, and its ~7e-4 error contribution is invisible next to fp8's). Per
(o, tb) accumulation group that's 6 DR + 20 bf16 instructions instead of 32,
a 0.81x PE-time ratio at ~1.83e-2 relative error (gate: 2e-2).
All matmuls share one PSUM group: fp8 weights are scaled by 512 for e4m3
range (Wfull values ~0.02 would be subnormal), bf16 weights are pre-scaled by
the same 512 (exact: power of 2), and the drain rescales by 1/512 while adding
the bias.
"""

import numpy as np
import ml_dtypes
import concourse.bacc as bacc
import concourse.mybir as mybir
import concourse.tile as tile
from concourse.bass_utils import run_bass_kernel_spmd

# problem shapes (hardcoded per contract)
B, S = 2, 2048
IN_FEATURES = 4096
OUT_FEATURES = 4096
NTOKENS = B * S                  # 4096
E4 = ml_dtypes.float8_e4m3
BF = ml_dtypes.bfloat16

NCORES = 8
TG, OG = 4, 2                    # token groups x out-feature groups
T = NTOKENS // TG                # 1024 tokens per core
O = OUT_FEATURES // OG           # 2048 out features per core
P = 128
KT = IN_FEATURES // P            # 32 contraction tiles
OT = O // P                      # 16 out-feature tiles per core
NTOK = 512                       # moving free dim per matmul
TB = T // NTOK                   # 2 token blocks per core

NF8 = 12                         # k-tiles computed in fp8 DoubleRow
NPAIR = NF8 // 2                 # DR instructions per (o, tb): 6
K8 = NF8 * P                     # 1536 fp8 contraction features
KR = KT - NF8                    # 20 fp32r k-tiles
SW = 512.0                       # fp8 weight scale (power of 2: exact)
INV_S = 1.0 / SW

F32R = mybir.dt.float32r
BF16 = mybir.dt.bfloat16
F32 = mybir.dt.float32
F8 = mybir.dt.float8e4
DR = mybir.MatmulPerfMode.DoubleRow
IDENT = mybir.ActivationFunctionType.Identity

# knobs for test.py
TRACE = False
LAST_RESULTS = None


def round_fp32r(a: np.ndarray) -> np.ndarray:
    """Round fp32 to the nearest fp32r-representable value (11-bit mantissa)."""
    u = np.ascontiguousarray(a, dtype=np.float32).view(np.uint32)
    r = (u.astype(np.uint64) + 0x7FF + ((u >> 12) & 1)) & 0xFFFFF000
    return r.astype(np.uint32).view(np.float32)


WCHUNK = 4        # fp32r k-tiles per W DMA
KC = KR // WCHUNK  # 5 fp32r W chunks per o-group
WBUFS = 32        # W chunk pool bufs


def build_nc(repeats: int = 1):
    nc = bacc.Bacc()
    # fp8 xT pair slabs: [pair][128, 2, TB*NTOK]
    x8p = nc.dram_tensor("x8p", [NPAIR, P, 2, TB * NTOK], F8, kind="ExternalInput")
    # fp32r xT slabs: [k][128, TB*NTOK]
    xw = nc.dram_tensor("xw", [KR, P, TB * NTOK], BF16, kind="ExternalInput")
    # fp8 W: [o][pair, 128, plane, 128]
    w8 = nc.dram_tensor("w8", [OT, NPAIR, P, 2, P], F8, kind="ExternalInput")
    # fp32r W chunked [o][kc][WCHUNK, 128, 128]
    wrest = nc.dram_tensor(
        "wrest", [OT, KC, WCHUNK, P, P], BF16, kind="ExternalInput"
    )
    # bias in o-partition layout: [128, OT]
    bo = nc.dram_tensor("bo", [P, OT], F32, kind="ExternalInput")
    out = nc.dram_tensor("out", [OT, TB, P, NTOK], F32, kind="ExternalOutput")

    NWARM = 4  # o-groups processed k-major while the xT stream arrives

    with tile.TileContext(nc) as tc:
        with (
            tc.tile_pool(name="xw_sb", bufs=1) as xw_sb,
            tc.tile_pool(name="w_sb", bufs=WBUFS) as w_sb,
            tc.tile_pool(name="w8_sb", bufs=8) as w8_sb,
            tc.tile_pool(name="o_sb", bufs=6) as o_sb,
            tc.tile_pool(name="ps", bufs=8, space="PSUM") as ps,
        ):
            bo_t = xw_sb.tile([P, OT], F32, tag="bo")

            # PE HAM warmup: dummy matmuls on memset data fill the dead time
            # while the first DMAs land, so real matmuls start at 2.4 GHz
            dummy_sb = xw_sb.tile([P, NTOK], F32R, tag="dummy")
            nc.vector.memset(dummy_sb.bitcast(F32), 0.0)
            ps_d = ps.tile([P, NTOK], F32, tag="ps", name="ps_dummy")
            for _ in range(11):
                nc.tensor.matmul(
                    ps_d, dummy_sb[:, :P], dummy_sb, start=True, stop=True
                )

            wts = {}
            w8s = {}

            def load_w8(o, rep, eng=None):
                w8t = w8_sb.tile(
                    [P, NPAIR, 2, P], F8, tag="w8t", name=f"w8t_{rep}_{o}"
                )
                (eng or nc.sync).dma_start(
                    out=w8t, in_=w8[o].rearrange("s k t c -> k s t c")
                )
                w8s[o] = w8t

            def load_w(o, rep):
                load_w8(o, rep)
                for kc in range(KC):
                    wt = w_sb.tile(
                        [P, WCHUNK, P], BF16, tag="wt", name=f"wt_{rep}_{o}_{kc}"
                    )
                    # dram [WCHUNK, 128, 128] -> sbuf [128, WCHUNK, 128];
                    # alternate issue queues to halve SP issue bursts
                    eng = nc.sync if kc % 2 == 0 else nc.scalar
                    eng.dma_start(
                        out=wt, in_=wrest[o, kc].rearrange("c k o -> k c o")
                    )
                    wts[o, kc] = wt

            # fp8 stream first (small, lands fast): w8 tiles for the warmup
            # groups on SP, x8 pair slabs on the activation queue
            x8_t = {}
            for o in range(NWARM):
                # split the warmup w8 loads across the SP and (cold, otherwise
                # idle) gpsimd queues so all four land ~2x sooner
                load_w8(o, 0, eng=nc.sync if o < 2 else nc.gpsimd)
                if o < NPAIR:
                    t = xw_sb.tile([P, 2, TB * NTOK], F8, tag=f"x8_{o}")
                    nc.scalar.dma_start(out=t, in_=x8p[o])
                    x8_t[o] = t
            for p_ in range(NWARM, NPAIR):
                t = xw_sb.tile([P, 2, TB * NTOK], F8, tag=f"x8_{p_}")
                nc.scalar.dma_start(out=t, in_=x8p[p_])
                x8_t[p_] = t

            # fp32r stream: W chunks for the warmup groups interleave with xT
            # slabs in warmup consumption order (k-major)
            xw_t = {}
            for kc in range(KC):
                for o in range(NWARM):
                    wt = w_sb.tile(
                        [P, WCHUNK, P], BF16, tag="wt", name=f"wt_0_{o}_{kc}"
                    )
                    nc.sync.dma_start(
                        out=wt, in_=wrest[o, kc].rearrange("c k o -> k c o")
                    )
                    wts[o, kc] = wt
                    k = kc * WCHUNK + o
                    if o < WCHUNK:
                        t = xw_sb.tile([P, TB * NTOK], BF16, tag=f"xw_{k}")
                        nc.scalar.dma_start(out=t, in_=xw[k])
                        xw_t[k] = t
                for k in range(kc * WCHUNK, (kc + 1) * WCHUNK):
                    if k not in xw_t:
                        t = xw_sb.tile([P, TB * NTOK], BF16, tag=f"xw_{k}")
                        nc.scalar.dma_start(out=t, in_=xw[k])
                        xw_t[k] = t
                if kc == 0:
                    # bias load is only needed by the drains, ~60us later;
                    # keep its issue slot off the critical input queues
                    nc.gpsimd.dma_start(out=bo_t, in_=bo[:, :])

            def drain(o, tb, psum):
                o_t = o_sb.tile([P, NTOK], F32, tag="ot", name=f"ot_{o}_{tb}")
                # psum -> sbuf rescaling 1/512 with per-partition bias add;
                # alternate engines so consecutive drains run in parallel
                if (o * TB + tb) % 2 == 0:
                    nc.scalar.activation(
                        o_t, psum, IDENT, bias=bo_t[:, o : o + 1], scale=INV_S
                    )
                else:
                    nc.vector.tensor_scalar(
                        o_t, psum, INV_S, bo_t[:, o : o + 1],
                        op0=mybir.AluOpType.mult, op1=mybir.AluOpType.add,
                    )
                # out DMAs ride the otherwise-idle gpsimd queue, EXCEPT the
                # last group's: gpsimd's final dge_drain takes ~4us, so its
                # queue must go quiet before the kernel tail
                if o == OT - 1:
                    eng = nc.scalar if tb == 0 else nc.sync
                else:
                    eng = nc.gpsimd
                eng.dma_start(out=out[o, tb, :, :], in_=o_t)

            def mm_group(o, rep):
                psums = {
                    tb: ps.tile([P, NTOK], F32, tag="ps", name=f"ps_{rep}_{o}_{tb}")
                    for tb in range(TB)
                }
                if o == OT - 1 or o == NWARM:
                    # tb-sequential groups: the last one so tb0's drain + out
                    # DMA overlap tb1's matmul chain (shorter kernel tail);
                    # the first steady one so tb0 runs on the spare (dummy)
                    # psum bank while the warmup drains are still freeing
                    # banks for tb1
                    for tb in range(TB):
                        for p_ in range(NPAIR):
                            nc.tensor.matmul(
                                psums[tb],
                                w8s[o][:, p_],
                                x8_t[p_][:, :, tb * NTOK : (tb + 1) * NTOK],
                                start=(p_ == 0),
                                stop=False,
                                perf_mode=DR,
                            )
                        for k in range(KR):
                            lhsT = wts[o, k // WCHUNK][:, k % WCHUNK]
                            nc.tensor.matmul(
                                psums[tb],
                                lhsT,
                                xw_t[k][:, tb * NTOK : (tb + 1) * NTOK],
                                start=False,
                                stop=(k == KR - 1),
                            )
                        if o == OT - 1 and tb == TB - 1:
                            # final drain split in half across both compute
                            # engines + both free DMA queues: the first out
                            # bytes leave ~0.4us after the last matmul
                            o_t = o_sb.tile(
                                [P, NTOK], F32, tag="ot", name="ot_final"
                            )
                            h = NTOK // 2
                            nc.scalar.activation(
                                o_t[:, :h], psums[tb][:, :h], IDENT,
                                bias=bo_t[:, o : o + 1], scale=INV_S,
                            )
                            nc.vector.tensor_scalar(
                                o_t[:, h:], psums[tb][:, h:], INV_S,
                                bo_t[:, o : o + 1],
                                op0=mybir.AluOpType.mult,
                                op1=mybir.AluOpType.add,
                            )
                            nc.sync.dma_start(
                                out=out[o, tb, :, :h], in_=o_t[:, :h]
                            )
                            nc.scalar.dma_start(
                                out=out[o, tb, :, h:], in_=o_t[:, h:]
                            )
                        else:
                            drain(o, tb, psums[tb])
                    return
                for p_ in range(NPAIR):
                    lhsT = w8s[o][:, p_]
                    for tb in range(TB):
                        nc.tensor.matmul(
                            psums[tb],
                            lhsT,
                            x8_t[p_][:, :, tb * NTOK : (tb + 1) * NTOK],
                            start=(p_ == 0),
                            stop=False,
                            perf_mode=DR,
                        )
                for k in range(KR):
                    lhsT = wts[o, k // WCHUNK][:, k % WCHUNK]
                    for tb in range(TB):
                        nc.tensor.matmul(
                            psums[tb],
                            lhsT,
                            xw_t[k][:, tb * NTOK : (tb + 1) * NTOK],
                            start=False,
                            stop=(k == KR - 1),
                        )
                for tb in range(TB):
                    drain(o, tb, psums[tb])

            for _rep in range(repeats):
                if _rep == 0:
                    # warmup phase: k-major over NWARM o-groups x TB token
                    # blocks (all 8 psum banks) -> 8 matmuls per arriving
                    # slab, keeping the PE busy while x streams in. fp8
                    # pair slabs run first (they land first).
                    psums = {
                        (o, tb): ps.tile(
                            [P, NTOK], F32, tag="ps", name=f"psw_{o}_{tb}"
                        )
                        for o in range(NWARM)
                        for tb in range(TB)
                    }
                    # DR phase in diagonal (o+p) waves: cell (o, p) needs
                    # w8[o] (arriving ~1.4us apart on two cold queues) and
                    # x8 pair p (~1.3us apart on a third) -- the wave order
                    # consumes cells roughly in arrival order, so the PE
                    # stays fed during the DMA-ring cold start
                    for s_ in range(NWARM + NPAIR - 1):
                        for o in range(NWARM):
                            p_ = s_ - o
                            if not (0 <= p_ < NPAIR):
                                continue
                            lhsT = w8s[o][:, p_]
                            for tb in range(TB):
                                nc.tensor.matmul(
                                    psums[o, tb],
                                    lhsT,
                                    x8_t[p_][:, :, tb * NTOK : (tb + 1) * NTOK],
                                    start=(p_ == 0),
                                    stop=False,
                                    perf_mode=DR,
                                )
                    for k in range(KR - WCHUNK):
                        for o in range(NWARM):
                            lhsT = wts[o, k // WCHUNK][:, k % WCHUNK]
                            for tb in range(TB):
                                nc.tensor.matmul(
                                    psums[o, tb],
                                    lhsT,
                                    xw_t[k][:, tb * NTOK : (tb + 1) * NTOK],
                                    start=False,
                                    stop=False,
                                )
                    # last k-window o-major with immediate drains, so psum
                    # banks free one o-group at a time and the steady phase
                    # starts while the rest of the warmup finishes
                    for o in range(NWARM):
                        for k in range(KR - WCHUNK, KR):
                            lhsT = wts[o, k // WCHUNK][:, k % WCHUNK]
                            for tb in range(TB):
                                nc.tensor.matmul(
                                    psums[o, tb],
                                    lhsT,
                                    xw_t[k][:, tb * NTOK : (tb + 1) * NTOK],
                                    start=False,
                                    stop=(k == KR - 1),
                                )
                        for tb in range(TB):
                            drain(o, tb, psums[o, tb])
                    o_start = NWARM
                else:
                    o_start = 0
                for o in range(o_start, OT):
                    load_w(o, _rep)
                    mm_group(o, _rep)
    nc.finalize()
    return nc


_NC = None


def _get_nc():
    global _NC
    if _NC is None:
        _NC = build_nc()
    return _NC


def _build_wfull(weights, input_indices, output_indices):
    """Wfull[k, o] = sum over blocks/dups of weights[n, j, i]."""
    ii = np.asarray(input_indices).astype(np.int64)     # [NBLK, BI]
    oi = np.asarray(output_indices).astype(np.int64)    # [NBLK, BO]
    w = np.asarray(weights, dtype=np.float64)           # [NBLK, BO, BI]
    flat = (ii[:, :, None] * OUT_FEATURES + oi[:, None, :]).ravel()  # [n, i, j]
    vals = np.ascontiguousarray(np.swapaxes(w, 1, 2)).ravel()        # [n, i, j]
    wfull = np.bincount(flat, weights=vals, minlength=IN_FEATURES * OUT_FEATURES)
    return wfull.reshape(IN_FEATURES, OUT_FEATURES).astype(np.float32)


def prepare_in_maps(x, weights, bias, input_indices, output_indices):
    x = np.asarray(x, dtype=np.float32)
    bias = np.asarray(bias, dtype=np.float32)

    wfull = _build_wfull(weights, input_indices, output_indices)
    x2 = x.reshape(NTOKENS, IN_FEATURES)

    # fp8 region (k < K8): e4m3 inputs, weights scaled by SW for e4m3 range
    x8full = x2[:, :K8].astype(E4)                            # [tok, K8]
    w8full = (wfull[:K8] * SW).astype(E4)                     # [K8, out]
    # fp32r region, weights pre-scaled by SW so one PSUM group shares the
    # 1/SW drain rescale (power-of-2: exact)
    xrfull = x2[:, K8:].astype(BF)                            # [tok, KR*P]
    wrfull = (wfull[K8:] * SW).astype(BF)                     # [KR*P, out]

    in_maps = []
    for c in range(NCORES):
        tg, og = divmod(c, OG)
        tok = slice(tg * T, (tg + 1) * T)
        osl = slice(og * O, (og + 1) * O)
        # fp8 xT pair slabs [pair, 128, plane, T]
        x8T = np.ascontiguousarray(x8full[tok].T)             # [K8, T]
        x8c = np.ascontiguousarray(
            x8T.reshape(NPAIR, 2, P, T).transpose(0, 2, 1, 3)
        )
        # fp32r xT slabs [k, 128, T]
        xT = np.ascontiguousarray(xrfull[tok].T)              # [KR*P, T]
        xwc = np.ascontiguousarray(xT.reshape(KR, P, T))
        # fp8 W [o, pair, 128, plane, 128]
        w8c = np.ascontiguousarray(
            w8full[:, osl].reshape(NPAIR, 2, P, OT, P).transpose(3, 0, 2, 1, 4)
        )
        # fp32r W [o, kc, WCHUNK, 128, 128]
        wr = np.ascontiguousarray(
            wrfull[:, osl].reshape(KR, P, OT, P).transpose(2, 0, 1, 3)
        ).reshape(OT, KC, WCHUNK, P, P)
        # bias in o-partition layout [128, OT]; full fp32 (added at drain)
        boc = np.ascontiguousarray(bias[osl].reshape(OT, P).T)
        in_maps.append(
            {"x8p": x8c, "xw": xwc, "w8": w8c, "wrest": wr, "bo": boc}
        )
    return in_maps


def assemble_output(core_outs):
    full = np.empty((NTOKENS, OUT_FEATURES), np.float32)
    for c in range(NCORES):
        tg, og = divmod(c, OG)
        o4 = np.asarray(core_outs[c])                    # [OT, TB, P, NTOK]
        blk = o4.transpose(1, 3, 0, 2).reshape(T, O)     # [t, o]
        full[tg * T : (tg + 1) * T, og * O : (og + 1) * O] = blk
    return full.reshape(B, S, OUT_FEATURES)


def kernel(x, weights, bias, input_indices, output_indices):
    global LAST_RESULTS
    in_maps = prepare_in_maps(x, weights, bias, input_indices, output_indices)
    nc = _get_nc()
    res = run_bass_kernel_spmd(nc, in_maps, list(range(NCORES)))
    LAST_RESULTS = res
    return assemble_output([res.results[c]["out"] for c in range(NCORES)])
